# revision 59
# baseline (speedup 1.0000x reference)
"""Trainium2 Bass kernel for GQA attention (B=2, S=2048, HID=1024, 16 q / 4 kv
heads, HD=64, RoPE, causal softmax).

Sharding: 8 cores = 2 batches x 4 kv-head groups. Each core computes its
(batch, kv-group) shard end-to-end and writes a partial output projection;
the host sums the 4 partials per batch.

v3 design notes:
  - all matmul operands bf16 (PSUM stays f32). The scalar engine (exp) is
    the roofline at ~95us; everything else is scheduled to hide under it.
  - single fused stream: proj(0) upfront, proj(1..3) interleaved into the
    first attention chunk's t-loop (flush barriers before each key-tile
    group keep emission order correct), out-projections sprinkled into
    later chunks. Attention chunk order 2,3,1,0 front-loads scalar-engine
    work and leaves the smallest chunk (and tail) for last.
  - causal masking: full-block -1e9 handled by partial-width score/exp;
    the within-block triangle is zeroed POST-exp on the Pool engine
    (multiply by a 0/1 mask), keeping the tensor engine free of mask work.
  - softmax denominators: ones-column in V gives sums for free; reciprocal
    on DVE (approx_fast, full-partition range), broadcast across partitions
    via a DRAM-roundtrip stride-0 DMA, scale on DVE. h2=1 rows reach
    attnT[64:128] via an SBUF->SBUF DMA (engines cannot cross partitions).
  - PSUM budget (8 banks): scores 2x[128,2,512] (4) + pv 2x[65,512] (2) +
    shared proj/rope/vt/outproj pool 2x[128,512] (2).
"""
import sys

sys.path.insert(0, "/opt/trn_rl_repo")

import numpy as np
import ml_dtypes
from contextlib import ExitStack

import concourse.bass as bass
import concourse.tile as tile
from concourse import bacc
from concourse import mybir
from concourse.bass_utils import run_bass_kernel_spmd

# problem constants (hardcoded per contract)
B, S, HID = 2, 2048, 1024
NH, NKV, HD = 16, 4, 64
P = 128
NK = HID // P          # 8 k-tiles over hidden
NQC = S // 512         # 4 q-chunks of 512
NST = S // P           # 16 seq tiles of 128
QH = NH // NKV         # 4 q heads per core
FEAT = QH * HD         # 256 features per core
MASK_NEG = -1e9
CHUNK_ORDER = [1, 2, 3, 0]

F32 = mybir.dt.float32
F32R = mybir.dt.float32r
BF16 = mybir.dt.bfloat16


def _pin_act_tables():
    # Make every activation-table set except natural_log_exp_and_others
    # ineligible so Exp/Copy resolve to one table and bacc hoists a single
    # ACT_TABLE_LOAD.
    import concourse.hw_specs as hw_specs
    import concourse.bacc as bacc_mod
    real = hw_specs.get_activation_tables

    def pinned(arch):
        tabs = dict(real(arch))
        return {name: (funcs if name == "natural_log_exp_and_others" else set())
                for name, funcs in tabs.items()}

    bacc_mod.get_activation_tables = pinned


def build_program(debug_dump=False):
    _pin_act_tables()
    nc = bacc.Bacc("TRN2", target_bir_lowering=False, debug=False)
    if debug_dump:
        d_dbg_q = nc.dram_tensor("dbg_q", [P, 2, S], BF16, kind="ExternalOutput").ap()
        d_dbg_k = nc.dram_tensor("dbg_k", [P, S], BF16, kind="ExternalOutput").ap()
        d_dbg_v = nc.dram_tensor("dbg_v", [P, NST, HD + 1], BF16, kind="ExternalOutput").ap()
        d_dbg_a = nc.dram_tensor("dbg_a", [P, 2, S], BF16, kind="ExternalOutput").ap()

    d_xT = nc.dram_tensor("xT", [HID, S], BF16, kind="ExternalInput").ap()
    d_wqT = nc.dram_tensor("wqT", [HID, FEAT], BF16, kind="ExternalInput").ap()
    d_wkvT = nc.dram_tensor("wkvT", [HID, 2 * HD], BF16, kind="ExternalInput").ap()
    d_woT = nc.dram_tensor("woT", [FEAT, HID], BF16, kind="ExternalInput").ap()
    d_cosT = nc.dram_tensor("cosT", [P, S], BF16, kind="ExternalInput").ap()
    d_sinT = nc.dram_tensor("sinT", [P, S], BF16, kind="ExternalInput").ap()
    d_ident = nc.dram_tensor("ident", [P, HD], BF16, kind="ExternalInput").ap()
    d_tri01 = nc.dram_tensor("tri01", [P, 2, P], BF16, kind="ExternalInput").ap()
    d_rscr = nc.dram_tensor("rscr", [32, 512], F32, kind="Internal").ap()
    d_onesv = nc.dram_tensor("onesv", [P, NST], BF16, kind="ExternalInput").ap()
    d_out = nc.dram_tensor("outp", [S, HID], BF16, kind="ExternalOutput").ap()

    with tile.TileContext(nc) as tc, ExitStack() as ctx, \
            nc.allow_low_precision(reason="2e-2 rel tolerance; bf16 throughout"):
        consts = ctx.enter_context(tc.tile_pool(name="consts", bufs=1))
        main = ctx.enter_context(tc.tile_pool(name="main", bufs=1))

        # ---- constants to SBUF: everything on the scalar queue (idle until
        # the first exp), in consumption-priority order; sync+gpsimd carry x
        wq_sb = consts.tile([P, NK, FEAT], BF16)
        nc.scalar.dma_start(wq_sb[:], d_wqT.rearrange("(ko p) m -> p ko m", p=P))
        cos_sb = consts.tile([P, S], BF16)
        nc.scalar.dma_start(cos_sb[:], d_cosT)
        sin_sb = consts.tile([P, S], BF16)
        nc.scalar.dma_start(sin_sb[:], d_sinT)
        wkv_sb = consts.tile([P, NK, 2 * HD], BF16)
        nc.scalar.dma_start(wkv_sb[:], d_wkvT.rearrange("(ko p) m -> p ko m", p=P))
        vaug = main.tile([P, NST, HD + 1], BF16)  # V seq-major + ones column
        nc.scalar.dma_start(
            vaug[:, :, HD:HD + 1],
            d_onesv.rearrange("p (n o) -> p n o", o=1))
        ident_sb = consts.tile([P, HD], BF16)
        nc.scalar.dma_start(ident_sb[:], d_ident)
        tri01_sb = consts.tile([P, 2, P], BF16)
        nc.scalar.dma_start(tri01_sb[:], d_tri01)
        wo_sb = consts.tile([P, 2, HID], BF16)
        nc.scalar.dma_start(wo_sb[:], d_woT.rearrange("(ko p) m -> p ko m", p=P))

        # ---- persistent activations (bf16)
        qpt = main.tile([P, 2, S], BF16)     # roped Q^T; tile m: heads 2m,2m+1
        kpt = main.tile([P, S], BF16)        # roped K^T duplicated to both halves
        attnT = main.tile([P, 2, S], BF16)   # normalized attention, feature-major

        # ---- pools
        xpool = ctx.enter_context(tc.tile_pool(name="xt", bufs=16))
        rawp = ctx.enter_context(tc.tile_pool(name="raw", bufs=3))
        tmpp = ctx.enter_context(tc.tile_pool(name="ropetmp", bufs=4))
        ptp = ctx.enter_context(tc.tile_pool(name="pt", bufs=4))
        recp = ctx.enter_context(tc.tile_pool(name="rec", bufs=2))
        stg = ctx.enter_context(tc.tile_pool(name="stg", bufs=2))
        osb = ctx.enter_context(tc.tile_pool(name="osb", bufs=3))
        scps = ctx.enter_context(tc.tile_pool(name="scps", bufs=2, space="PSUM"))
        pvps = ctx.enter_context(tc.tile_pool(name="pvps", bufs=1, space="PSUM"))
        pa = ctx.enter_context(tc.tile_pool(name="pa", bufs=2, space="PSUM"))

        # ---------------- projection pieces (per chunk) ----------------
        x_tiles = {}   # chunk -> list of xt tiles

        def p_dma_x(n):
            def go():
                tiles = []
                engs = [nc.sync, nc.gpsimd]
                for k in range(NK):
                    xt = xpool.tile([P, 512], BF16)
                    engs[k % 2].dma_start(
                        xt[:], d_xT[k * P:(k + 1) * P, n * 512:(n + 1) * 512])
                    tiles.append(xt)
                x_tiles[n] = tiles
            return go

        def p_burst_piece(n, part, k0):
            # 2 of the 8 k-step matmuls; small pieces keep the scalar-engine
            # pipeline fed while projections stream through the t-loop
            def go():
                if k0 == 0:
                    x_tiles[(n, part, "ps")] = pa.tile(
                        [P, 512], F32, tag="pa", name=f"pj{n}{part}")
                ps = x_tiles[(n, part, "ps")]
                for k in (k0, k0 + 1):
                    if part == 0:
                        w = wq_sb[:, k, 0:P]
                    elif part == 1:
                        w = wq_sb[:, k, P:FEAT]
                    else:
                        w = wkv_sb[:, k, :]
                    nc.tensor.matmul(ps[:], w, x_tiles[n][k][:],
                                     start=(k == 0), stop=(k == NK - 1),
                                     skip_group_check=True)
                if k0 == NK - 2:
                    raw = rawp.tile([P, 512], BF16, tag="raw",
                                    name=f"rw{n}{part}")
                    nc.vector.tensor_copy(raw[:], ps[:])
                    x_tiles[(n, part)] = raw
            return go

        # RoPE without the tensor engine: rotate-half is a partition shift
        # (cross-partition SBUF->SBUF DMA); the sign lives in the sin table
        # (rows 0:32 of each 64-block negated host-side).
        def p_rope_a(n, part):
            def go():
                raw = x_tiles[(n, part)]
                rot = tmpp.tile([P, 512], BF16, tag="rot",
                                name=f"rot{n}{part}")
                if part == 2:   # K: duplicate to both halves while rotating
                    k2 = tmpp.tile([P, 512], BF16, tag="k2", name=f"k2{n}")
                    for half in (0, 1):
                        b = half * HD
                        nc.gpsimd.dma_start(k2[b:b + HD, :], raw[0:HD, :])
                        nc.gpsimd.dma_start(rot[b:b + 32, :], raw[32:HD, :])
                        nc.gpsimd.dma_start(rot[b + 32:b + HD, :],
                                            raw[0:32, :])
                    x_tiles[(n, part, "k2")] = k2
                else:
                    for h in (0, 1):
                        b = h * HD
                        nc.gpsimd.dma_start(rot[b:b + 32, :],
                                            raw[b + 32:b + HD, :])
                        nc.gpsimd.dma_start(rot[b + 32:b + HD, :],
                                            raw[b:b + 32, :])
                x_tiles[(n, part, "rot")] = rot
            return go

        def p_rope_b(n, part):
            def go():
                c0 = n * 512
                cs = cos_sb[:, c0:c0 + 512]
                sn = sin_sb[:, c0:c0 + 512]
                base = x_tiles[(n, part, "k2")] if part == 2 \
                    else x_tiles[(n, part)]
                t1 = tmpp.tile([P, 512], BF16, tag="t1")
                nc.vector.tensor_mul(t1[:], base[:], cs)
                t2 = tmpp.tile([P, 512], BF16, tag="t2")
                nc.vector.tensor_mul(t2[:], x_tiles[(n, part, "rot")][:], sn)
                if part == 2:
                    nc.gpsimd.tensor_add(kpt[:, c0:c0 + 512], t1[:], t2[:])
                else:
                    nc.gpsimd.tensor_add(qpt[:, part, c0:c0 + 512],
                                         t1[:], t2[:])
            return go

        def p_vt(n, tt):
            def go():
                kvraw = x_tiles[(n, 2)]
                st = 4 * n + tt
                ps = pa.tile([P, 512], F32, tag="pa", name=f"vt{n}{tt}")
                vview = ps[:, 0:HD // 2].bitcast(BF16)
                nc.tensor.transpose(vview, kvraw[HD:P, tt * P:(tt + 1) * P],
                                    ident_sb[HD:P, :])
                nc.vector.tensor_copy(vaug[:, st, 0:HD], vview)
            return go

        def proj_pieces(n):
            ps = []
            for part in (0, 1, 2):
                ps += [p_burst_piece(n, part, k0) for k0 in range(0, NK, 2)]
                ps.append(p_rope_a(n, part))
                ps.append(p_rope_b(n, part))
            ps += [p_vt(n, tt) for tt in range(4)]
            return ps

        # ---------------- out-projection pieces ----------------
        def p_outproj(j, st4, nn):
            def go():
                st = 4 * j + st4
                po = pa.tile([P, 512], F32, tag="pa", name=f"po{j}{st4}{nn}")
                for m in range(2):
                    nc.tensor.matmul(po[:],
                                     attnT[:, m, st * P:(st + 1) * P],
                                     wo_sb[:, m, nn * 512:(nn + 1) * 512],
                                     start=(m == 0), stop=(m == 1))
                ot = osb.tile([P, 512], BF16, tag="ot")
                # late chunks' copies land after the last exps -> scalar
                # engine has tail slack there; stores stay off gpsimd so the
                # drain DMAs there are never queued behind them
                if j in (CHUNK_ORDER[2], CHUNK_ORDER[3]):
                    nc.scalar.copy(ot[:], po[:])
                    engs = [nc.sync, nc.scalar]
                else:
                    nc.vector.tensor_copy(ot[:], po[:])
                    engs = [nc.sync]
                engs[(st4 * 2 + nn) % len(engs)].dma_start(
                    d_out[st * P:(st + 1) * P, nn * 512:(nn + 1) * 512],
                    ot[:])
            return go

        # ---------------- piece queue with flush barriers ----------------
        queue = []            # pending (closure, marker) pairs
        projected = set()     # markers: ('q', n) / ('kv', n) fully emitted

        def flush_until(marker):
            while marker not in projected:
                piece, meta = queue.pop(0)
                piece()
                if meta is not None:
                    projected.add(meta)

        def pop_piece():
            if queue:
                piece, meta = queue.pop(0)
                piece()
                if meta is not None:
                    projected.add(meta)

        # seed: x for chunks 0/1 + proj(0) dense upfront; later chunks'
        # pieces stream through the attention t-loops (flush barriers keep
        # emission order correct)
        p_dma_x(0)()
        p_dma_x(1)()
        for pc in proj_pieces(0):
            pc()
        projected.add(0)
        queue.append((p_dma_x(2), None))
        for n in (1, 2, 3):
            ps = proj_pieces(n)
            if n == 2:
                queue.append((p_dma_x(3), None))
            for pc in ps[:-1]:
                queue.append((pc, None))
            queue.append((ps[-1], n))   # last piece marks chunk complete

        # ---------------- attention ----------------
        drain_idx = [0]

        def attn_chunk(j):
            c0 = j * 512
            T = 4 * j + 4
            flush_until(j)          # Q of chunk j needed from the first score
            for m in range(2):
                sc_tiles = {}
                pt_tiles = {}

                def emit_score(t):
                    # all K/V tiles for this key block must be on-chip
                    flush_until(t // 4)
                    r = t - 4 * j
                    lo = P * r if r >= 0 else 0
                    sc = scps.tile([P, 2, 512], F32, tag="sc")
                    sc_tiles[t] = (sc, lo)
                    for h2 in (0, 1):
                        half = h2 * HD
                        kl = kpt[half:half + HD, t * P:(t + 1) * P]
                        ql = qpt[half:half + HD, m, c0 + lo:c0 + 512]
                        nc.tensor.matmul(sc[:, h2, lo:512], kl, ql,
                                         start=True, stop=True,
                                         skip_group_check=True)

                def emit_exp(t):
                    sc, lo = sc_tiles[t]
                    pt = ptp.tile([P, 2, 512], BF16, tag="ptt")
                    pt_tiles[t] = (pt, lo)
                    nc.scalar.activation(
                        pt[:, :, lo:512], sc[:, :, lo:512],
                        mybir.ActivationFunctionType.Exp, scale=0.125)
                    r = t - 4 * j
                    if r >= 0:
                        # zero the within-block upper triangle post-exp
                        nc.gpsimd.tensor_mul(pt[:, :, lo:lo + P],
                                             pt[:, :, lo:lo + P], tri01_sb[:])

                def emit_pv(t, pvh):
                    # two matmuls: a PSUM accumulation group cannot span the
                    # two banks the h2 halves live in
                    pt, lo = pt_tiles.pop(t)
                    sc_tiles.pop(t)
                    for h2 in (0, 1):
                        nc.tensor.matmul(
                            pvh[0:HD + 1, h2, lo:512],
                            vaug[:, t, :],
                            pt[:, h2, lo:512],
                            start=(t == 0), stop=(t == T - 1),
                            skip_group_check=True)

                pvh = pvps.tile([HD + 1, 2, 512], F32, tag="pv",
                                name=f"pv{m}{j}")
                emit_score(0)
                emit_exp(0)
                for t in range(T):
                    if t + 1 < T:
                        emit_score(t + 1)
                        emit_exp(t + 1)
                    # interleave work lands where PV(t) would stall on exp(t)
                    pop_piece()
                    emit_pv(t, pvh)

                # drain: denominators -> SBUF, batched recip, DRAM-roundtrip
                # partition broadcast, scale, h2=1 partition shift via DMA
                slot = drain_idx[0]
                drain_idx[0] += 2
                dn = recp.tile([HD + 1, 2, 512], F32, tag="dn")
                nc.vector.tensor_copy(dn[:], pvh[:])
                recr = recp.tile([HD + 1, 2, 512], F32, tag="recr")
                nc.vector.reciprocal_approx_fast(recr[:], dn[:])
                nc.gpsimd.dma_start(d_rscr[slot:slot + 2, :],
                                    recr[HD:HD + 1, :, :])
                rec_s = recp.tile([HD, 2, 512], F32, tag="recs")
                for h2 in (0, 1):
                    nc.gpsimd.dma_start(
                        rec_s[:, h2, :],
                        d_rscr[slot + h2:slot + h2 + 1, :]
                        .partition_broadcast(HD))
                nc.vector.tensor_mul(attnT[0:HD, m, c0:c0 + 512],
                                     dn[0:HD, 0, :], rec_s[:, 0, :])
                sh = stg.tile([HD, 512], BF16, tag="sh")
                nc.vector.tensor_mul(sh[:], dn[0:HD, 1, :], rec_s[:, 1, :])
                nc.gpsimd.dma_start(attnT[HD:P, m, c0:c0 + 512], sh[:])

        for j in CHUNK_ORDER:
            attn_chunk(j)
            for st4 in range(4):
                for nn in range(2):
                    queue.append((p_outproj(j, st4, nn), None))
        while queue:
            pop_piece()

        if debug_dump:
            nc.sync.dma_start(d_dbg_q, qpt[:])
            nc.sync.dma_start(d_dbg_k, kpt[:])
            nc.sync.dma_start(d_dbg_v, vaug[:])
            nc.sync.dma_start(d_dbg_a, attnT[:])

    nc.compile()
    return nc


def make_consts():
    """Host-precomputed constant operands shared by all cores."""
    bf = ml_dtypes.bfloat16
    ident = np.zeros((P, HD), np.float32)
    ident[HD:P, :] = np.eye(HD)
    # 0/1 keep-mask for the within-block lower triangle, dup'd over h2
    t01 = (np.arange(P)[:, None] <= np.arange(P)[None, :]).astype(np.float32)
    tri01 = np.stack([t01, t01], axis=1).astype(bf)   # [P, 2, P]
    return dict(ident=ident.astype(bf), tri01=tri01,
                onesv=np.ones((P, NST), bf))


def make_in_maps(x, cos, sin, wq, wk, wv, wo):
    """Per-core input tensors (host-side layout prep, bf16)."""
    bf = ml_dtypes.bfloat16
    consts = make_consts()
    cosT = np.ascontiguousarray(np.vstack([cos.T, cos.T])).astype(bf)
    sinT = np.ascontiguousarray(np.vstack([sin.T, sin.T]))
    sinT[0:HD // 2] *= -1.0
    sinT[HD:HD + HD // 2] *= -1.0
    sinT = sinT.astype(bf)
    in_maps = []
    for core in range(8):
        b, g = core // NKV, core % NKV
        xT = np.ascontiguousarray(x[b].T.astype(bf))
        wqT = np.ascontiguousarray(wq[g * FEAT:(g + 1) * FEAT, :].T.astype(bf))
        wkvT = np.ascontiguousarray(
            np.concatenate([wk[g * HD:(g + 1) * HD, :],
                            wv[g * HD:(g + 1) * HD, :]], axis=0).T.astype(bf))
        woT = np.ascontiguousarray(wo[:, g * FEAT:(g + 1) * FEAT].T.astype(bf))
        in_maps.append(dict(xT=xT, wqT=wqT, wkvT=wkvT, woT=woT,
                            cosT=cosT, sinT=sinT, **consts))
    return in_maps


_PROG = None


def kernel(x, cos, sin, wq, wk, wv, wo):
    global _PROG
    x = np.asarray(x, np.float32)
    cos = np.asarray(cos, np.float32)
    sin = np.asarray(sin, np.float32)
    wq = np.asarray(wq, np.float32)
    wk = np.asarray(wk, np.float32)
    wv = np.asarray(wv, np.float32)
    wo = np.asarray(wo, np.float32)

    in_maps = make_in_maps(x, cos, sin, wq, wk, wv, wo)

    if _PROG is None:
        _PROG = build_program()
    res = run_bass_kernel_spmd(_PROG, in_maps, core_ids=list(range(8)))

    out = np.zeros((B, S, HID), np.float32)
    for core in range(8):
        out[core // NKV] += np.asarray(res.results[core]["outp"], np.float32)
    return out


if __name__ == "__main__":
    rng = np.random.default_rng(0)
    ins = dict(
        x=rng.standard_normal((B, S, HID)).astype(np.float32),
        cos=rng.random((S, HD)).astype(np.float32),
        sin=rng.random((S, HD)).astype(np.float32),
        wq=(rng.standard_normal((HID, HID)) * HID ** -0.5).astype(np.float32),
        wk=(rng.standard_normal((NKV * HD, HID)) * HID ** -0.5).astype(np.float32),
        wv=(rng.standard_normal((NKV * HD, HID)) * HID ** -0.5).astype(np.float32),
        wo=(rng.standard_normal((HID, HID)) * HID ** -0.5).astype(np.float32),
    )
    out = kernel(**ins)
    print("kernel ran, out shape", out.shape, "mean", float(np.abs(out).mean()))
